# revision 12
# baseline (speedup 1.0000x reference)
"""Trainium2 Bass kernel for nn_EnhancedTokenwiseAggregation.

Computes, for stream_outputs [S,B,L,E] (S=4 streams as KV, a single shared
query), a per-token multi-head attention pooling:
    aggregated [B,L,E], weights [S,B,L] (head-mean attn probabilities).

Strategy (per the sharding hint): shard the B*L=16384 tokens across 8
NeuronCores (2048 tokens each); replicate the tiny projection weights.

Algebraic folding done on the host (all exact in fp32 math):
  - logits[tok,h,s] = <wl[h,:], x[tok,s,:]> where wl = (q @ Wq^T + bq)
    reshaped [H,D], scaled by 1/sqrt(D), contracted with Wk. The bk bias
    shifts all s equally per head, so softmax cancels it (dropped).
  - v-bias bv contributes exactly Wo @ bv to the output (softmax sums to 1
    over s), so it is folded into the output bias.
Device pipeline per 128-token chunk (feature-major x via DMA-transpose):
  logits (PE) -> exp (ACT) -> sum/recip (DVE) -> V-proj (PE, PSUM-accum)
  -> attn-weighted stream sum (DVE scalar_tensor_tensor FMA, per-partition
  scalars) -> ctx transpose (PE) -> out-proj + bias (PE) -> DMA out.

Inputs are cast to bf16 on the host; matmuls run in bf16 with fp32 PSUM
accumulation (measured ~3e-3 max rel err vs the fp32 reference).
"""

import numpy as np
import ml_dtypes
from contextlib import ExitStack

import concourse.bass as bass
import concourse.bacc as bacc
import concourse.mybir as mybir
import concourse.tile as tile
from concourse.bass_utils import run_bass_kernel_spmd

S, B, L, E, H = 4, 4, 4096, 1024, 8
D = E // H                 # 128
NCORES = 8
N = B * L                  # 16384 tokens
NTOK = N // NCORES         # 2048 tokens per core
TBLK = 512                 # tokens per DMA-transpose block
TCH = 128                  # tokens per matmul chunk (PE stationary width)
EC = E // 128              # 8 contraction chunks
BF = mybir.dt.bfloat16
F32 = mybir.dt.float32
FH = 512                   # matmul output free-dim half (PSUM bank limit)

AX = mybir.AxisListType
OP = mybir.AluOpType
ACTF = mybir.ActivationFunctionType


def build_kernel_body(ctx: ExitStack, tc: "tile.TileContext", ntok: int):
    nc = tc.nc
    x = nc.dram_tensor("x", [S, ntok, E], BF, kind="ExternalInput")
    wvt = nc.dram_tensor("wvt", [E, E], BF, kind="ExternalInput")
    wot = nc.dram_tensor("wot", [E, E], BF, kind="ExternalInput")
    wlt = nc.dram_tensor("wlt", [E, H], BF, kind="ExternalInput")
    bo = nc.dram_tensor("bo", [1, E], BF, kind="ExternalInput")
    ident = nc.dram_tensor("ident", [128, 128], BF, kind="ExternalInput")
    identf = nc.dram_tensor("identf", [H, H], F32, kind="ExternalInput")
    agg = nc.dram_tensor("agg", [ntok, E], F32, kind="ExternalOutput")
    wout = nc.dram_tensor("wout", [ntok, S], F32, kind="ExternalOutput")

    consts = ctx.enter_context(tc.tile_pool(name="consts", bufs=1))
    wv_sb = consts.tile([128, EC, E], BF, tag="wv")
    nc.sync.dma_start(wv_sb[:], wvt[:].rearrange("(ec p) f -> p ec f", p=128))
    wo_sb = consts.tile([128, EC, E], BF, tag="wo")
    nc.sync.dma_start(wo_sb[:], wot[:].rearrange("(ec p) f -> p ec f", p=128))
    wl_sb = consts.tile([128, EC, H], BF, tag="wl")
    nc.sync.dma_start(wl_sb[:], wlt[:].rearrange("(ec p) h -> p ec h", p=128))
    bo_sb = consts.tile([1, E], BF, tag="bo")
    nc.sync.dma_start(bo_sb[:], bo[:])
    id_sb = consts.tile([128, 128], BF, tag="id")
    nc.sync.dma_start(id_sb[:], ident[:])
    idf_sb = consts.tile([H, H], F32, tag="idf")
    nc.sync.dma_start(idf_sb[:], identf[:])
    ones_sb = consts.tile([1, 128], BF, tag="ones")
    nc.vector.memset(ones_sb[:], 1.0)

    xt_pool = ctx.enter_context(tc.tile_pool(name="xt", bufs=2))
    lgt_pool = ctx.enter_context(tc.tile_pool(name="lgt", bufs=2))
    attn_pool = ctx.enter_context(tc.tile_pool(name="attn", bufs=3))
    ctx_pool = ctx.enter_context(tc.tile_pool(name="ctx", bufs=2))
    ctt_pool = ctx.enter_context(tc.tile_pool(name="ctt", bufs=2))
    agg_sb_pool = ctx.enter_context(tc.tile_pool(name="aggsb", bufs=2))
    small_pool = ctx.enter_context(tc.tile_pool(name="small", bufs=3))

    lg_psum = ctx.enter_context(tc.tile_pool(name="lg", bufs=1, space="PSUM"))
    v_psum = ctx.enter_context(tc.tile_pool(name="v", bufs=2, space="PSUM"))
    ct_psum = ctx.enter_context(tc.tile_pool(name="ct", bufs=1, space="PSUM"))
    agg_psum = ctx.enter_context(tc.tile_pool(name="ag", bufs=1, space="PSUM"))

    nblk = ntok // TBLK
    for tb in range(nblk):
        xt_all = xt_pool.tile([128, S, EC, TBLK], BF, tag="xt")
        for s in range(S):
            # one xbar-transpose DMA per stream: [TBLK, E] -> [128, EC, TBLK]
            nc.sync.dma_start_transpose(
                out=xt_all[:, s], in_=x[s, tb * TBLK:(tb + 1) * TBLK, :]
            )
        xts = [xt_all[:, s] for s in range(S)]

        for t in range(TBLK // TCH):
            tok = bass.ts(t, TCH)
            row0 = tb * TBLK + t * TCH

            # ---- logits, head-major: lgT[h, s, tok] = <wl[h], x[tok,s]> ----
            # wl is the tiny stationary (8-row LDW); all 4 streams ride one
            # matmul via a strided rhs over xt_all.
            lgt = lg_psum.tile([H, S, TCH], F32, tag="lg")
            for e in range(EC):
                nc.tensor.matmul(
                    lgt[:],
                    lhsT=wl_sb[:, e, :],
                    rhs=xt_all[:, :, e, tok],
                    start=(e == 0),
                    stop=(e == EC - 1),
                )
            lgt_sb = lgt_pool.tile([H, S, TCH], F32, tag="lgt")
            nc.scalar.copy(lgt_sb[:], lgt[:])
            # transpose to token-major [tok, s*H+h] (fp32 identity)
            lg = lg_psum.tile([128, S * H], F32, tag="lg")
            for s in range(S):
                nc.tensor.transpose(
                    lg[:, s * H:(s + 1) * H], lgt_sb[:, s, :], idf_sb[:]
                )

            # ---- softmax over s (logits are O(1); no max subtraction) ----
            attn = attn_pool.tile([128, H, S], F32, tag="attn")
            nc.scalar.activation(
                attn[:], lg[:].rearrange("p (s h) -> p h s", h=H), ACTF.Exp
            )
            z = small_pool.tile([128, H], F32, tag="z")
            nc.vector.tensor_reduce(z[:], attn[:], axis=AX.X, op=OP.add)
            rz = small_pool.tile([128, H], F32, tag="rz")
            nc.vector.reciprocal(rz[:], z[:])
            attn2 = attn_pool.tile([128, H, S], F32, tag="attn2")
            for h in range(H):
                nc.vector.tensor_scalar_mul(
                    attn2[:, h, :], attn[:, h, :], rz[:, h:h + 1]
                )

            # ---- head-mean attn -> weights output ----
            wsum = small_pool.tile([128, S], F32, tag="wsum")
            nc.vector.tensor_reduce(
                wsum[:], attn2[:].rearrange("p h s -> p s h"), axis=AX.X, op=OP.add
            )
            wo_t = small_pool.tile([128, S], F32, tag="wot")
            nc.scalar.mul(wo_t[:], wsum[:], 1.0 / H)
            nc.sync.dma_start(wout[row0:row0 + TCH, :], wo_t[:])

            # ---- V-projection per stream + attn-weighted accumulation ----
            cpp = None
            for s in range(S):
                v = v_psum.tile([128, E], F32, tag="v")
                for e in range(EC):
                    for fh in range(E // FH):
                        nc.tensor.matmul(
                            v[:, fh * FH:(fh + 1) * FH],
                            lhsT=xts[s][:, e, tok],
                            rhs=wv_sb[:, e, fh * FH:(fh + 1) * FH],
                            start=(e == 0),
                            stop=(e == EC - 1),
                        )
                cnew = ctx_pool.tile([128, E], BF, tag="ctx")
                for h in range(H):
                    hd = bass.ts(h, D)
                    if s == 0:
                        nc.scalar.mul(cnew[:, hd], v[:, hd], attn2[:, h, 0:1])
                    else:
                        nc.vector.scalar_tensor_tensor(
                            out=cnew[:, hd],
                            in0=v[:, hd],
                            scalar=attn2[:, h, s:s + 1],
                            in1=cpp[:, hd],
                            op0=OP.mult,
                            op1=OP.add,
                        )
                cpp = cnew

            # ---- transpose ctx to feature-major ----
            ct = ct_psum.tile([128, E], BF, tag="ct")
            for e in range(EC):
                nc.tensor.transpose(
                    ct[:, e * 128:(e + 1) * 128], cpp[:, e * 128:(e + 1) * 128],
                    id_sb[:],
                )
            ctt = ctt_pool.tile([128, E], BF, tag="ctt")
            nc.scalar.copy(ctt[:], ct[:])

            # ---- out-projection + bias ----
            ag = agg_psum.tile([128, E], F32, tag="ag")
            for fh in range(E // FH):
                fsl = bass.ts(fh, FH)
                for e in range(EC):
                    nc.tensor.matmul(
                        ag[:, fsl],
                        lhsT=ctt[:, e * 128:(e + 1) * 128],
                        rhs=wo_sb[:, e, fsl],
                        start=(e == 0),
                        stop=False,
                    )
                nc.tensor.matmul(
                    ag[:, fsl], lhsT=ones_sb[:], rhs=bo_sb[:, fsl],
                    start=False, stop=True,
                )
            ag_sb = agg_sb_pool.tile([128, E], F32, tag="aggsb")
            nc.scalar.copy(ag_sb[:], ag[:])
            nc.sync.dma_start(agg[row0:row0 + TCH, :], ag_sb[:])


def build_nc(ntok: int = NTOK) -> bass.Bass:
    nc = bacc.Bacc("TRN2", target_bir_lowering=False, debug=False)
    with tile.TileContext(nc) as tc, ExitStack() as ctx:
        build_kernel_body(ctx, tc, ntok)
    nc.compile()
    return nc


def host_prep(stream_query, in_proj_weight, in_proj_bias, out_proj_weight,
              out_proj_bias):
    """Fold q/Wk into logit weights, bv into the output bias; cast to bf16."""
    Wq, Wk = in_proj_weight[:E], in_proj_weight[E:2 * E]
    Wv = in_proj_weight[2 * E:]
    bq, bv = in_proj_bias[:E], in_proj_bias[2 * E:]
    q = (stream_query[0, 0] @ Wq.T + bq).reshape(H, D) / np.float32(np.sqrt(D))
    wl = np.einsum(
        "hd,hde->he", q.astype(np.float64),
        Wk.reshape(H, D, E).astype(np.float64),
    ).astype(np.float32)                                   # [H, E]
    bo_eff = out_proj_bias + out_proj_weight @ bv          # [E]
    b16 = ml_dtypes.bfloat16
    return {
        "wvt": np.ascontiguousarray(Wv.T).astype(b16),     # [E, E] (e, f)
        "wot": np.ascontiguousarray(out_proj_weight.T).astype(b16),
        "wlt": np.ascontiguousarray(wl.T).astype(b16),     # [E, H]
        "bo": bo_eff.reshape(1, E).astype(b16),
        "ident": np.eye(128, dtype=b16),
        "identf": np.eye(H, dtype=np.float32),
    }


_NC_CACHE = {}


def _get_nc(ntok):
    if ntok not in _NC_CACHE:
        _NC_CACHE[ntok] = build_nc(ntok)
    return _NC_CACHE[ntok]


def kernel(stream_outputs, stream_query, in_proj_weight, in_proj_bias,
           out_proj_weight, out_proj_bias):
    assert stream_outputs.shape == (S, B, L, E)
    shared = host_prep(stream_query, in_proj_weight, in_proj_bias,
                       out_proj_weight, out_proj_bias)
    xr = stream_outputs.reshape(S, N, E).astype(ml_dtypes.bfloat16)
    in_maps = []
    for c in range(NCORES):
        m = dict(shared)
        m["x"] = np.ascontiguousarray(xr[:, c * NTOK:(c + 1) * NTOK, :])
        in_maps.append(m)

    nc = _get_nc(NTOK)
    res = run_bass_kernel_spmd(nc, in_maps, list(range(NCORES)))

    agg = np.concatenate([res.results[c]["agg"] for c in range(NCORES)], axis=0)
    wts = np.concatenate([res.results[c]["wout"] for c in range(NCORES)], axis=0)
    aggregated = agg.reshape(B, L, E)
    weights = np.ascontiguousarray(wts.T.reshape(S, B, L))
    return aggregated, weights


# revision 17
# speedup vs baseline: 1.2961x; 1.2961x over previous
"""Trainium2 Bass kernel for nn_EnhancedTokenwiseAggregation.

Computes, for stream_outputs [S,B,L,E] (S=4 streams as KV, a single shared
query), a per-token multi-head attention pooling:
    aggregated [B,L,E], weights [S,B,L] (head-mean attn probabilities).

Strategy (per the sharding hint): shard the B*L=16384 tokens across 8
NeuronCores (2048 tokens each); replicate the tiny projection weights.

Algebraic folding done on the host (all exact in fp32 math):
  - logits[tok,h,s] = <wl[h,:], x[tok,s,:]> where wl = (q @ Wq^T + bq)
    reshaped [H,D], scaled by 1/sqrt(D), contracted with Wk. The bk bias
    shifts all s equally per head, so softmax cancels it (dropped).
  - v-bias bv contributes exactly Wo @ bv to the output (softmax sums to 1
    over s), so it is folded into the output bias.
Device pipeline per 128-token chunk (feature-major x via DMA-transpose):
  logits (PE) -> exp (ACT) -> sum/recip (DVE) -> V-proj (PE, PSUM-accum)
  -> attn-weighted stream sum (DVE scalar_tensor_tensor FMA, per-partition
  scalars) -> ctx transpose (PE) -> out-proj + bias (PE) -> DMA out.

Inputs are cast to bf16 on the host; matmuls run in bf16 with fp32 PSUM
accumulation (measured ~3e-3 max rel err vs the fp32 reference).
"""

import numpy as np
import ml_dtypes
from contextlib import ExitStack

import concourse.bass as bass
import concourse.bacc as bacc
import concourse.mybir as mybir
import concourse.tile as tile
from concourse.bass_utils import run_bass_kernel_spmd

S, B, L, E, H = 4, 4, 4096, 1024, 8
D = E // H                 # 128
NCORES = 8
N = B * L                  # 16384 tokens
NTOK = N // NCORES         # 2048 tokens per core
TBLK = 512                 # tokens per DMA-transpose block
TCH = 128                  # tokens per matmul chunk (PE stationary width)
EC = E // 128              # 8 contraction chunks
BF = mybir.dt.bfloat16
F32 = mybir.dt.float32
FH = 512                   # matmul output free-dim half (PSUM bank limit)

AX = mybir.AxisListType
OP = mybir.AluOpType
ACTF = mybir.ActivationFunctionType


def build_kernel_body(ctx: ExitStack, tc: "tile.TileContext", ntok: int):
    nc = tc.nc
    x = nc.dram_tensor("x", [S, ntok, E], BF, kind="ExternalInput")
    wvt = nc.dram_tensor("wvt", [E, E], BF, kind="ExternalInput")
    wot = nc.dram_tensor("wot", [E, E], BF, kind="ExternalInput")
    wlt = nc.dram_tensor("wlt", [E, H], BF, kind="ExternalInput")
    bo = nc.dram_tensor("bo", [1, E], BF, kind="ExternalInput")
    ident = nc.dram_tensor("ident", [128, 128], BF, kind="ExternalInput")
    agg = nc.dram_tensor("agg", [ntok, E], F32, kind="ExternalOutput")
    wout = nc.dram_tensor("wout", [ntok, S], F32, kind="ExternalOutput")

    consts = ctx.enter_context(tc.tile_pool(name="consts", bufs=1))
    wv_sb = consts.tile([128, EC, E], BF, tag="wv")
    nc.sync.dma_start(wv_sb[:], wvt[:].rearrange("(ec p) f -> p ec f", p=128))
    wo_sb = consts.tile([128, EC, E], BF, tag="wo")
    nc.sync.dma_start(wo_sb[:], wot[:].rearrange("(ec p) f -> p ec f", p=128))
    wl_sb = consts.tile([128, EC, H], BF, tag="wl")
    nc.sync.dma_start(wl_sb[:], wlt[:].rearrange("(ec p) h -> p ec h", p=128))
    bo_sb = consts.tile([1, E], BF, tag="bo")
    nc.sync.dma_start(bo_sb[:], bo[:])
    id_sb = consts.tile([128, 128], BF, tag="id")
    nc.sync.dma_start(id_sb[:], ident[:])
    ones_sb = consts.tile([1, 128], BF, tag="ones")
    nc.vector.memset(ones_sb[:], 1.0)

    xt_pool = ctx.enter_context(tc.tile_pool(name="xt", bufs=2))
    attn_pool = ctx.enter_context(tc.tile_pool(name="attn", bufs=3))
    ctx_pool = ctx.enter_context(tc.tile_pool(name="ctx", bufs=2))
    ctt_pool = ctx.enter_context(tc.tile_pool(name="ctt", bufs=2))
    agg_sb_pool = ctx.enter_context(tc.tile_pool(name="aggsb", bufs=2))
    small_pool = ctx.enter_context(tc.tile_pool(name="small", bufs=3))

    lg_psum = ctx.enter_context(tc.tile_pool(name="lg", bufs=1, space="PSUM"))
    v_psum = ctx.enter_context(tc.tile_pool(name="v", bufs=2, space="PSUM"))
    ct_psum = ctx.enter_context(tc.tile_pool(name="ct", bufs=1, space="PSUM"))
    agg_psum = ctx.enter_context(tc.tile_pool(name="ag", bufs=1, space="PSUM"))

    nblk = ntok // TBLK
    for tb in range(nblk):
        xt_all = xt_pool.tile([128, S, EC, TBLK], BF, tag="xt")
        for s in range(S):
            # one xbar-transpose DMA per stream: [TBLK, E] -> [128, EC, TBLK]
            nc.sync.dma_start_transpose(
                out=xt_all[:, s], in_=x[s, tb * TBLK:(tb + 1) * TBLK, :]
            )
        xts = [xt_all[:, s] for s in range(S)]

        for t in range(TBLK // TCH):
            tok = bass.ts(t, TCH)
            row0 = tb * TBLK + t * TCH

            # ---- per stream: V-projection + logits (shared stationary),
            #      exp, and UNNORMALIZED weighted accumulation.
            #      Softmax normalization (1/Z) is applied after the loop,
            #      so each v_s PSUM tile is consumed immediately. ----
            attn_un = attn_pool.tile([128, H, S], F32, tag="attn")
            cpp = None
            for s in range(S):
                v = v_psum.tile([128, E], F32, tag="v")
                lg = lg_psum.tile([128, H], F32, tag="lg")
                for e in range(EC):
                    for fh in range(E // FH):
                        nc.tensor.matmul(
                            v[:, fh * FH:(fh + 1) * FH],
                            lhsT=xts[s][:, e, tok],
                            rhs=wv_sb[:, e, fh * FH:(fh + 1) * FH],
                            start=(e == 0),
                            stop=(e == EC - 1),
                        )
                    nc.tensor.matmul(
                        lg[:],
                        lhsT=xts[s][:, e, tok],
                        rhs=wl_sb[:, e, :],
                        start=(e == 0),
                        stop=(e == EC - 1),
                    )
                # e_hs = exp(logit) (logits are O(1); no max subtraction)
                nc.scalar.activation(attn_un[:, :, s], lg[:], ACTF.Exp)
                cnew = ctx_pool.tile([128, E], BF, tag="ctx")
                for h in range(H):
                    hd = bass.ts(h, D)
                    if s == 0:
                        nc.scalar.mul(cnew[:, hd], v[:, hd],
                                      attn_un[:, h, 0:1])
                    else:
                        nc.vector.scalar_tensor_tensor(
                            out=cnew[:, hd],
                            in0=v[:, hd],
                            scalar=attn_un[:, h, s:s + 1],
                            in1=cpp[:, hd],
                            op0=OP.mult,
                            op1=OP.add,
                        )
                cpp = cnew

            # ---- softmax normalizer ----
            z = small_pool.tile([128, H], F32, tag="z")
            nc.vector.tensor_reduce(z[:], attn_un[:], axis=AX.X, op=OP.add)
            rz = small_pool.tile([128, H], F32, tag="rz")
            nc.vector.reciprocal(rz[:], z[:])

            # ---- head-mean normalized attn -> weights output ----
            attn2 = attn_pool.tile([128, H, S], F32, tag="attn2")
            for h in range(H):
                nc.vector.tensor_scalar_mul(
                    attn2[:, h, :], attn_un[:, h, :], rz[:, h:h + 1]
                )
            wsum = small_pool.tile([128, S], F32, tag="wsum")
            nc.vector.tensor_reduce(
                wsum[:], attn2[:].rearrange("p h s -> p s h"), axis=AX.X,
                op=OP.add
            )
            wo_t = small_pool.tile([128, S], F32, tag="wot")
            nc.scalar.mul(wo_t[:], wsum[:], 1.0 / H)
            nc.sync.dma_start(wout[row0:row0 + TCH, :], wo_t[:])

            # ---- normalize ctx by 1/Z per head (split ACT/DVE) ----
            ctn = ctx_pool.tile([128, E], BF, tag="ctxn")
            for h in range(H):
                hd = bass.ts(h, D)
                if h % 2 == 0:
                    nc.scalar.mul(ctn[:, hd], cpp[:, hd], rz[:, h:h + 1])
                else:
                    nc.vector.tensor_scalar_mul(ctn[:, hd], cpp[:, hd],
                                                rz[:, h:h + 1])

            # ---- transpose ctx to feature-major ----
            ct = ct_psum.tile([128, E], BF, tag="ct")
            for e in range(EC):
                nc.tensor.transpose(
                    ct[:, e * 128:(e + 1) * 128], ctn[:, e * 128:(e + 1) * 128],
                    id_sb[:],
                )
            ctt = ctt_pool.tile([128, E], BF, tag="ctt")
            nc.scalar.copy(ctt[:], ct[:])

            # ---- out-projection + bias ----
            ag = agg_psum.tile([128, E], F32, tag="ag")
            for fh in range(E // FH):
                fsl = bass.ts(fh, FH)
                for e in range(EC):
                    nc.tensor.matmul(
                        ag[:, fsl],
                        lhsT=ctt[:, e * 128:(e + 1) * 128],
                        rhs=wo_sb[:, e, fsl],
                        start=(e == 0),
                        stop=False,
                    )
                nc.tensor.matmul(
                    ag[:, fsl], lhsT=ones_sb[:], rhs=bo_sb[:, fsl],
                    start=False, stop=True,
                )
            ag_sb = agg_sb_pool.tile([128, E], F32, tag="aggsb")
            nc.scalar.copy(ag_sb[:], ag[:])
            nc.sync.dma_start(agg[row0:row0 + TCH, :], ag_sb[:])


def build_nc(ntok: int = NTOK) -> bass.Bass:
    nc = bacc.Bacc("TRN2", target_bir_lowering=False, debug=False)
    with tile.TileContext(nc) as tc, ExitStack() as ctx:
        build_kernel_body(ctx, tc, ntok)
    nc.compile()
    return nc


def host_prep(stream_query, in_proj_weight, in_proj_bias, out_proj_weight,
              out_proj_bias):
    """Fold q/Wk into logit weights, bv into the output bias; cast to bf16."""
    Wq, Wk = in_proj_weight[:E], in_proj_weight[E:2 * E]
    Wv = in_proj_weight[2 * E:]
    bq, bv = in_proj_bias[:E], in_proj_bias[2 * E:]
    q = (stream_query[0, 0] @ Wq.T + bq).reshape(H, D) / np.float32(np.sqrt(D))
    wl = np.einsum(
        "hd,hde->he", q.astype(np.float64),
        Wk.reshape(H, D, E).astype(np.float64),
    ).astype(np.float32)                                   # [H, E]
    bo_eff = out_proj_bias + out_proj_weight @ bv          # [E]
    b16 = ml_dtypes.bfloat16
    return {
        "wvt": np.ascontiguousarray(Wv.T).astype(b16),     # [E, E] (e, f)
        "wot": np.ascontiguousarray(out_proj_weight.T).astype(b16),
        "wlt": np.ascontiguousarray(wl.T).astype(b16),     # [E, H]
        "bo": bo_eff.reshape(1, E).astype(b16),
        "ident": np.eye(128, dtype=b16),
    }


_NC_CACHE = {}


def _get_nc(ntok):
    if ntok not in _NC_CACHE:
        _NC_CACHE[ntok] = build_nc(ntok)
    return _NC_CACHE[ntok]


def kernel(stream_outputs, stream_query, in_proj_weight, in_proj_bias,
           out_proj_weight, out_proj_bias):
    assert stream_outputs.shape == (S, B, L, E)
    shared = host_prep(stream_query, in_proj_weight, in_proj_bias,
                       out_proj_weight, out_proj_bias)
    xr = stream_outputs.reshape(S, N, E).astype(ml_dtypes.bfloat16)
    in_maps = []
    for c in range(NCORES):
        m = dict(shared)
        m["x"] = np.ascontiguousarray(xr[:, c * NTOK:(c + 1) * NTOK, :])
        in_maps.append(m)

    nc = _get_nc(NTOK)
    res = run_bass_kernel_spmd(nc, in_maps, list(range(NCORES)))

    agg = np.concatenate([res.results[c]["agg"] for c in range(NCORES)], axis=0)
    wts = np.concatenate([res.results[c]["wout"] for c in range(NCORES)], axis=0)
    aggregated = agg.reshape(B, L, E)
    weights = np.ascontiguousarray(wts.T.reshape(S, B, L))
    return aggregated, weights
